# revision 47
# baseline (speedup 1.0000x reference)
"""Trainium2 Bass kernel for the MAMGCN encoder block.

Strategy: data-parallel over batch B=16 across 8 NeuronCores (2 batches/core).
Host-side prep (untimed): shard x, repack small weights, pre-transpose x to
(t*64+f, n) layout, cast matmul operands to bf16 (Vs to fp8e4m3). Device does
everything else: spatial attention (two fused weight matmuls -> product ->
tanh-sigmoid -> Vs@P -> exp -> column softmax), Chebyshev graph conv with
Theta folded in (Y = X @ Theta2 block-diag), matmuls in bf16 with fp32 PSUM
accumulation; the S = Vs@P matmul runs in fp8e4m3 with DoubleRow perf mode
(2 contraction rows per PE cell).

v6 (410.7us baseline -> ~391-399us measured):
- batched DMA: one descriptor per constant tensor (DRAM-side APs permuted
  so src/dst flat orders match), 3 per x batch, cheb as one [128,K,N] load
  per chunk, out stores staged per half-group on the sync queue
- 16 warm-up matmuls at kernel start so the attention phase runs at full
  PE clock (HAM unthrottled) as soon as x lands
- sphase in fp8e4m3 DoubleRow: 4 MMs of 256-contraction instead of 8 of
  128 (adds ~1.3e-3 rel err; conv must stay bf16 - e4m3 there measures
  3.8e-2, over the 2e-2 gate)
- bs folded into the product matmul via an identity-weight accumulate;
  tanh reads PSUM directly (keeps the DVE free to drain Y copies)
- Y built in cn-pairs into 2-bank PSUM tiles drained by one strided copy
  (the PSUM->SBUF copy rate is what paces the head phase)
- emission order = scheduler priority; each build_y follows the conv
  group that frees its y slot (pool-slot reuse order must match emission
  order or the shared rings stall/deadlock)
"""
import numpy as np
import ml_dtypes

B, N, F, T, K, FO = 16, 1024, 64, 24, 3, 64
NCORES = 8
BPC = B // NCORES          # batches per core
NCH = N // 128             # 8 partition chunks of N
NJ = (T * F) // 128        # 12 chunks of the tf dim
NG = 3                     # t-groups
TT = T // NG               # 8 t's per group
NJG = NJ // NG             # 4 tf-chunks per t-group
XG = 3                     # x DMA groups
NJX = NJ // XG             # 4 tf-chunks per x DMA
bf16 = ml_dtypes.bfloat16
f8e4 = ml_dtypes.float8_e4m3

_CACHE = {}


def _build_nc():
    import concourse.bacc as bacc
    import concourse.bass as bass
    import concourse.tile as tile
    import concourse.mybir as mybir

    fp32 = mybir.dt.float32
    bf = mybir.dt.bfloat16
    f8 = mybir.dt.float8e4
    AF = mybir.ActivationFunctionType
    DR = mybir.MatmulPerfMode.DoubleRow

    nc = bacc.Bacc(
        "TRN2", target_bir_lowering=False, debug=False,
        enable_asserts=True, num_devices=NCORES,
    )

    # ---- DRAM I/O ----
    x_d = nc.dram_tensor("x_tf", [BPC, NJ, 128, N], bf, kind="ExternalInput")
    bs_d = nc.dram_tensor("bs_t", [NCH, 128, N], bf, kind="ExternalInput")
    vs8_d = nc.dram_tensor("vs8_t", [NCH, 128, N], f8, kind="ExternalInput")
    cheb_d = nc.dram_tensor("cheb_t", [NCH, 128, K, N], bf,
                            kind="ExternalInput")
    wcat_d = nc.dram_tensor("wcat", [128, NJ, 2 * T], bf, kind="ExternalInput")
    th2_d = nc.dram_tensor("th2", [128, 2 * K * FO], bf, kind="ExternalInput")
    thf_d = nc.dram_tensor("thf", [128, K * FO], bf, kind="ExternalInput")
    hrow_d = nc.dram_tensor("hrow", [NCH, 128, 1], fp32, kind="ExternalInput")
    id_d = nc.dram_tensor("id128", [128, 128], bf, kind="ExternalInput")
    # out[b, group, mchunk, p, o, tl]
    out_d = nc.dram_tensor("out", [BPC, NG, NCH, 128, FO, TT], bf,
                           kind="ExternalOutput")

    with tile.TileContext(nc) as tc:
        with (
            tc.tile_pool(name="const", bufs=1) as cpool,
            tc.tile_pool(name="work", bufs=2) as wpool,
            tc.tile_pool(name="big", bufs=1) as bpool,
            tc.tile_pool(name="ypool", bufs=2) as ypool,
            tc.tile_pool(name="chpool", bufs=3) as chpool,
            tc.tile_pool(name="spool", bufs=2) as spool,
            tc.tile_pool(name="psA", bufs=2, space="PSUM") as psA,
            tc.tile_pool(name="psB", bufs=2, space="PSUM") as psB,
        ):
            # ---- constants ----
            wcat_sb = cpool.tile([128, NJ, 2 * T], bf, tag="wcat")
            th2_sb = cpool.tile([128, 2 * K * FO], bf, tag="th2")
            thf_sb = cpool.tile([128, K * FO], bf, tag="thf")
            vs8_sb = cpool.tile([128, NCH, N], f8, tag="vs8")
            bs_sb = cpool.tile([128, NCH, N], bf, tag="bs")
            hrow_sb = cpool.tile([128, NCH], fp32, tag="hrow")
            id_sb = cpool.tile([128, 128], bf, tag="id128")
            ones_sb = cpool.tile([128, 1], bf, tag="ones")
            one1_sb = cpool.tile([1, 1], fp32, tag="one1")
            warm_sb = cpool.tile([128, 512], bf, tag="warm")
            nc.gpsimd.memset(ones_sb[:], 1.0)
            nc.gpsimd.memset(one1_sb[:], 1.0)
            nc.gpsimd.memset(warm_sb[:], 0.25)

            # ---- PE warm-up: ~3.4us cold + a short warm tail so HAM
            # unthrottles right as x lands; more would gate the attention
            # matmuls behind the warm-up drain (results never consumed) ----
            for w in range(12):
                pw = psA.tile([128, 512], fp32, tag="big", name=f"warm{w}")
                nc.tensor.matmul(pw[:, :], warm_sb[:, 0:128], warm_sb[:, :],
                                 start=True, stop=True)

            def make_batch(b):
                st = {'ys': [], 'ch': [None] * NCH}

                def load_cheb(ic):
                    ch = chpool.tile([128, K, N], bf, tag="ch",
                                     name=f"ch{b}_{ic}")
                    st['ch'][ic] = ch
                    nc.sync.dma_start(ch[:], cheb_d[ic])

                def head_dma():
                    st['x'] = x_sb = bpool.tile([128, NJ, N], bf, tag="x",
                                                name=f"x{b}")
                    st['e'] = bpool.tile([128, NCH, N], bf, tag="e",
                                         name=f"e{b}")
                    st['p8'] = bpool.tile([128, NCH, N], f8, tag="p8",
                                          name=f"p{b}")
                    st['a'] = bpool.tile([128, K, NCH, N], bf, tag="a",
                                         name=f"a{b}")
                    st['rT'] = bpool.tile([128, NCH], fp32, tag="rT",
                                          name=f"rT{b}")
                    # DRAM-side APs rearranged so both sides' flattened dim
                    # orders match (DMA streams src sequence into dst sequence)
                    if b == 0:
                        nc.sync.dma_start(wcat_sb[:], wcat_d[:])
                    # first x group split in two so the attention matmuls
                    # start on the first half ~2us earlier
                    for h in range(2):
                        hj = NJX // 2
                        nc.sync.dma_start(
                            x_sb[:, h * hj:(h + 1) * hj, :],
                            x_d[b, h * hj:(h + 1) * hj].rearrange(
                                "j p n -> p j n"))
                    if b == 0:
                        nc.sync.dma_start(th2_sb[:], th2_d[:])
                        nc.sync.dma_start(thf_sb[:], thf_d[:])
                    for g in range(1, XG):
                        nc.sync.dma_start(
                            x_sb[:, g * NJX:(g + 1) * NJX, :],
                            x_d[b, g * NJX:(g + 1) * NJX].rearrange(
                                "j p n -> p j n"))
                    if b == 0:
                        nc.sync.dma_start(bs_sb[:],
                                          bs_d[:].rearrange("c p n -> p c n"))
                        nc.sync.dma_start(vs8_sb[:],
                                          vs8_d[:].rearrange("c p n -> p c n"))
                        nc.sync.dma_start(
                            hrow_sb[:],
                            hrow_d[:].rearrange("c p one -> p (c one)"))
                        nc.sync.dma_start(id_sb[:], id_d[:])
                    for ic in range(3):
                        load_cheb(ic)

                def att_group(g):
                    # attention pre-reductions (one pass over x), emitted per
                    # x DMA group so Y-build units back-fill the DMA gaps
                    x_sb = st['x']
                    if g == 0:
                        st['att_c'] = wpool.tile([2 * T, N], bf,
                                                 tag="attc", bufs=1,
                                                 name=f"attc{b}")
                        st['att_r'] = wpool.tile([T, N], bf, tag="attr",
                                                 bufs=1, name=f"attr{b}")
                        st['pas'] = [psA.tile([2 * T, 512], fp32, tag="big",
                                              name=f"pa{b}_{s}")
                                     for s in range(2)]
                    pas = st['pas']
                    for j in range(NJX * g, NJX * (g + 1)):
                        for s in range(2):
                            nc.tensor.matmul(
                                pas[s][:, :],
                                wcat_sb[:, j, :],
                                x_sb[:, j, s * 512:(s + 1) * 512],
                                start=(j == 0), stop=(j == NJ - 1),
                            )

                def att_fin():
                    att_c, att_r, pas = st['att_c'], st['att_r'], st['pas']
                    # copies split across DVE/ACT (parallel), shift split per
                    # half so each product half starts as soon as its half
                    # of att_r lands
                    nc.vector.tensor_copy(att_c[:, 0:512], pas[0][:])
                    nc.scalar.copy(att_c[:, 512:1024], pas[1][:])
                    for s in range(2):
                        nc.sync.dma_start(
                            att_r[:, s * 512:(s + 1) * 512],
                            att_c[T:2 * T, s * 512:(s + 1) * 512])

                def product():
                    # bs accumulated into the product PSUM via an identity
                    # matmul: extra PE work lands exactly in the window the
                    # PE would otherwise starve, and the DVE stays free to
                    # drain Y copies; tanh reads PSUM directly
                    att_c, att_r, p8 = st['att_c'], st['att_r'], st['p8']
                    for cn in range(NCH):
                        for s in range(2):
                            pp = psA.tile([128, 512], fp32, tag="big",
                                          name=f"pp{b}")
                            nc.tensor.matmul(
                                pp[:, :],
                                att_c[0:T, cn * 128:(cn + 1) * 128],
                                att_r[:, s * 512:(s + 1) * 512],
                                start=True, stop=False,
                            )
                            nc.tensor.matmul(
                                pp[:, :],
                                id_sb[:],
                                bs_sb[:, cn, s * 512:(s + 1) * 512],
                                start=False, stop=True,
                            )
                            nc.scalar.activation(
                                p8[:, cn, s * 512:(s + 1) * 512], pp[:],
                                AF.Tanh, scale=0.5)

                def y_pair(y_sb, j, tl0, cnp, eng):
                    # two Y units (adjacent cn) into one 2-bank PSUM tile,
                    # drained by a single copy: halves the per-element copy
                    # overhead that paces the whole head phase
                    x_sb = st['x']
                    cn0 = 2 * cnp
                    W = 2 * K * FO
                    py2 = psB.tile([128, 2, 512], fp32, tag="py2", bufs=2,
                                   name=f"py{b}")
                    for u in range(2):
                        csl = slice((cn0 + u) * 128, (cn0 + u + 1) * 128)
                        nc.tensor.matmul(
                            py2[:, u, 0:W], x_sb[:, j, csl], th2_sb[:],
                            start=True, stop=True,
                        )
                    src = py2[:, :, 0:W].rearrange(
                        "p u (k w) -> p u k w", k=K)
                    dst = y_sb[:, cn0:cn0 + 2, :, tl0:tl0 + 2, :].rearrange(
                        "p c k t o -> p c k (t o)")
                    if eng == 'd':
                        nc.vector.tensor_copy(dst, src)
                    else:
                        nc.scalar.copy(dst, src)

                def build_y(g, pat):
                    # pat: cycled per-pair copy-engine pattern, 'd'=DVE 'a'=ACT
                    y_sb = ypool.tile([128, NCH, K, TT, FO], bf, tag="y",
                                      name=f"y{b}_{g}")
                    st['ys'].append(y_sb)
                    i = 0
                    for j in range(NJG * g, NJG * (g + 1)):
                        for cnp in range(NCH // 2):
                            y_pair(y_sb, j, 2 * (j - NJG * g), cnp,
                                   pat[i % len(pat)])
                            i += 1

                def sphase():
                    e_sb, p8, a_sb = st['e'], st['p8'], st['a']
                    for ic in range(NCH):
                        for s in range(2):
                            ps = psA.tile([128, 512], fp32, tag="big",
                                          name=f"ps{b}")
                            for j2 in range(NCH // 2):
                                nc.tensor.matmul(
                                    ps[:, :],
                                    vs8_sb[:, 2 * j2:2 * j2 + 2,
                                           ic * 128:(ic + 1) * 128],
                                    p8[:, 2 * j2:2 * j2 + 2,
                                       s * 512:(s + 1) * 512],
                                    start=(j2 == 0), stop=(j2 == NCH // 2 - 1),
                                    perf_mode=DR,
                                )
                            nc.scalar.activation(
                                e_sb[:, ic, s * 512:(s + 1) * 512], ps[:],
                                AF.Exp, scale=0.5,
                                bias=hrow_sb[:, ic:ic + 1],
                            )
                        ch = st['ch'][ic]
                        for k in range(K):
                            nc.vector.tensor_mul(a_sb[:, k, ic, :],
                                                 ch[:, k, :],
                                                 e_sb[:, ic, :])
                        if ic + 3 < NCH:
                            load_cheb(ic + 3)

                def rt():
                    e_sb, rT_sb = st['e'], st['rT']
                    pcs = [psA.tile([1, 512], fp32, tag="big",
                                    name=f"pc{b}_{s}") for s in range(2)]
                    for ic in range(NCH):
                        for s in range(2):
                            nc.tensor.matmul(
                                pcs[s][:, :],
                                ones_sb[:],
                                e_sb[:, ic, s * 512:(s + 1) * 512],
                                start=(ic == 0), stop=(ic == NCH - 1),
                            )
                    csum_sb = wpool.tile([1, N], fp32, tag="csum_s", bufs=1,
                                         name=f"cs{b}")
                    for s in range(2):
                        nc.scalar.copy(csum_sb[:, s * 512:(s + 1) * 512],
                                       pcs[s][:])
                    prt = psA.tile([128, NCH], fp32, tag="big",
                                   name=f"prt{b}")
                    for c in range(NCH):
                        nc.tensor.matmul(
                            prt[:, c:c + 1],
                            csum_sb[:, c * 128:(c + 1) * 128],
                            one1_sb[:],
                            start=True, stop=True,
                        )
                    nc.vector.reciprocal(rT_sb[:], prt[:])

                def conv(g, tail=False):
                    a_sb, rT_sb, y_sb = st['a'], st['rT'], st['ys'][g]
                    for mh in range(2):
                        stage = spool.tile([128, NCH // 2, FO, TT], bf,
                                           tag="stage", name=f"st{b}")
                        fine = tail and mh == 1
                        for mi in range(NCH // 2):
                            mc = mh * (NCH // 2) + mi
                            po = psB.tile([128, TT, FO], fp32, tag="po",
                                          bufs=2, name=f"po{b}")
                            nmm = 0
                            for k in range(K):
                                for cn in range(NCH):
                                    nc.tensor.matmul(
                                        po[:, :, :],
                                        a_sb[:, k, cn,
                                             mc * 128:(mc + 1) * 128],
                                        y_sb[:, cn, k, :, :],
                                        start=(nmm == 0),
                                        stop=(nmm == K * NCH - 1),
                                    )
                                    nmm += 1
                            nc.scalar.activation(
                                stage[:, mi],
                                po[:, :, :].rearrange("p t o -> p o t"),
                                AF.Relu,
                                scale=rT_sb[:, mc:mc + 1],
                            )
                            if fine:
                                # kernel tail: ship each mc as soon as its
                                # relu lands instead of waiting for the
                                # whole half-group
                                nc.sync.dma_start(out_d[b, g, mc],
                                                  stage[:, mi])
                        if fine:
                            continue
                        # store on the sync queue (idle mid-kernel); issuing
                        # from ACT would stall the relu copies that recycle
                        # conv's PSUM slots
                        nc.sync.dma_start(
                            out_d[b, g, mh * (NCH // 2):(mh + 1) * (NCH // 2)]
                            .rearrange("m p o t -> p m o t"),
                            stage[:])

                st['head_dma'] = head_dma
                st['att_group'] = att_group
                st['att_fin'] = att_fin
                st['product'] = product
                st['build_y'] = build_y
                st['sphase'] = sphase
                st['rt'] = rt
                st['conv'] = conv
                return st

            # ---- emission order (= scheduler priority); the priority heap
            # back-fills any stall with ready lower-priority work ----
            # y slots (2) and the shared 6-slot PSUM ring are recycled in
            # emission order, so each build_y is emitted after the conv
            # that releases its slot; the priority heap back-fills stalls
            # with whatever is ready.
            b0 = make_batch(0)
            b0['head_dma']()
            b0['att_group'](0)
            b0['build_y'](0, 'da')
            b0['att_group'](1)
            b0['att_group'](2)
            b0['att_fin']()
            b0['product']()
            b0['build_y'](1, 'dda')   # PE filler under the add/tanh chain
            b0['sphase']()
            b0['rt']()
            b1 = make_batch(1)
            b1['head_dma']()
            # b1's attention emitted ahead of conv0 so its attc/attr chain
            # completes under the conv instead of serializing after it
            for g in range(XG):
                b1['att_group'](g)
            b1['att_fin']()
            b0['conv'](0)
            b0['build_y'](2, 'd')
            b0['conv'](1)
            b1['product']()
            b1['build_y'](0, 'd')
            b0['conv'](2)
            b1['sphase']()
            b1['build_y'](1, 'da')    # split drain: DVE is full of a-muls
            b1['rt']()
            b1['conv'](0)
            b1['build_y'](2, 'd')
            b1['conv'](1)
            b1['conv'](2, tail=True)

    nc.compile()
    return nc


def _host_prep(x, W1, W2, W3, bs, Vs, cheb, Theta):
    x = np.asarray(x, np.float32)
    W1 = np.asarray(W1, np.float32)
    W2 = np.asarray(W2, np.float32)
    W3 = np.asarray(W3, np.float32)
    bs = np.asarray(bs, np.float32)
    Vs = np.asarray(Vs, np.float32)
    cheb = np.asarray(cheb, np.float32)
    Theta = np.asarray(Theta, np.float32)

    x_tf = np.ascontiguousarray(x.transpose(0, 3, 2, 1)).reshape(B, NJ, 128, N)
    x_tf = x_tf.astype(bf16)
    bs_t = bs[0].reshape(NCH, 128, N).astype(bf16)
    vs8_t = np.ascontiguousarray(Vs.T).reshape(NCH, 128, N).astype(f8e4)
    cheb_t = np.ascontiguousarray(
        cheb.reshape(K, NCH, 128, N).transpose(1, 2, 0, 3)).astype(bf16)
    t_idx = np.arange(T * F) // F
    f_idx = np.arange(T * F) % F
    wl_flat = W1[t_idx][:, None] * W2[f_idx, :]
    wr_flat = np.zeros((T * F, T), np.float32)
    wr_flat[np.arange(T * F), t_idx] = W3[f_idx]
    wcat = np.concatenate([wl_flat, wr_flat], axis=1)
    # partition-major so the single DMA moves 2.3KB contiguous per partition
    wcat = np.ascontiguousarray(
        wcat.reshape(NJ, 128, 2 * T).transpose(1, 0, 2)).astype(bf16)
    # columns ordered (k, t-parity, o) so each Y pair drains with one
    # strided copy
    th2 = np.zeros((128, 2 * K * FO), np.float32)
    for par in range(2):
        for k in range(K):
            c0 = k * 2 * FO + par * FO
            th2[par * F:(par + 1) * F, c0:c0 + FO] = Theta[k]
    th2 = th2.astype(bf16)
    # f-contraction variant: both partition halves hold Theta, columns (k, o)
    thf = np.zeros((128, K * FO), np.float32)
    for par in range(2):
        for k in range(K):
            thf[par * F:(par + 1) * F, k * FO:(k + 1) * FO] = Theta[k]
    thf = thf.astype(bf16)
    hrow = (0.5 * Vs.sum(axis=1)).astype(np.float32).reshape(NCH, 128, 1)
    id128 = np.eye(128, dtype=np.float32).astype(bf16)
    return x_tf, bs_t, vs8_t, cheb_t, wcat, th2, thf, hrow, id128


def kernel(x, W1, W2, W3, bs, Vs, cheb, Theta, _return_results=False,
           _trace=False):
    from concourse.bass_utils import run_bass_kernel_spmd

    x_tf, bs_t, vs8_t, cheb_t, wcat, th2, thf, hrow, id128 = _host_prep(
        x, W1, W2, W3, bs, Vs, cheb, Theta)

    if "nc" not in _CACHE:
        _CACHE["nc"] = _build_nc()
    nc = _CACHE["nc"]

    shared = dict(bs_t=bs_t, vs8_t=vs8_t, cheb_t=cheb_t, wcat=wcat,
                  th2=th2, thf=thf, hrow=hrow, id128=id128)
    in_maps = []
    for c in range(NCORES):
        m = dict(shared)
        m["x_tf"] = np.ascontiguousarray(x_tf[c * BPC:(c + 1) * BPC])
        in_maps.append(m)

    _CACHE["in_maps"] = in_maps
    kw = {"trace": True} if _trace else {}
    res = run_bass_kernel_spmd(nc, in_maps, list(range(NCORES)), **kw)
    outs = []
    for c in range(NCORES):
        o = res.results[c]["out"]  # (BPC, NG, NCH, 128, FO, TT)
        o = np.asarray(o, np.float32)
        o = o.transpose(0, 2, 3, 4, 1, 5).reshape(BPC, N, FO, T)
        outs.append(o)
    full = np.concatenate(outs, axis=0).astype(np.float32)
    if _return_results:
        return full, res
    return full


# revision 49
# speedup vs baseline: 1.0097x; 1.0097x over previous
"""Trainium2 Bass kernel for the MAMGCN encoder block.

Strategy: data-parallel over batch B=16 across 8 NeuronCores (2 batches/core).
Host-side prep (untimed): shard x, repack small weights, pre-transpose x to
(t*64+f, n) layout, cast matmul operands to bf16 (Vs to fp8e4m3). Device does
everything else: spatial attention (two fused weight matmuls -> product ->
tanh-sigmoid -> Vs@P -> exp -> column softmax), Chebyshev graph conv with
Theta folded in (Y = X @ Theta2 block-diag), matmuls in bf16 with fp32 PSUM
accumulation; the S = Vs@P matmul runs in fp8e4m3 with DoubleRow perf mode
(2 contraction rows per PE cell).

v6 (410.7us baseline -> ~391-399us measured):
- batched DMA: one descriptor per constant tensor (DRAM-side APs permuted
  so src/dst flat orders match), 3 per x batch, cheb as one [128,K,N] load
  per chunk, out stores staged per half-group on the sync queue
- 16 warm-up matmuls at kernel start so the attention phase runs at full
  PE clock (HAM unthrottled) as soon as x lands
- sphase in fp8e4m3 DoubleRow: 4 MMs of 256-contraction instead of 8 of
  128 (adds ~1.3e-3 rel err; conv must stay bf16 - e4m3 there measures
  3.8e-2, over the 2e-2 gate)
- bs folded into the product matmul via an identity-weight accumulate;
  tanh reads PSUM directly (keeps the DVE free to drain Y copies)
- Y built in cn-pairs into 2-bank PSUM tiles drained by one strided copy
  (the PSUM->SBUF copy rate is what paces the head phase)
- emission order = scheduler priority; each build_y follows the conv
  group that frees its y slot (pool-slot reuse order must match emission
  order or the shared rings stall/deadlock)
"""
import numpy as np
import ml_dtypes

B, N, F, T, K, FO = 16, 1024, 64, 24, 3, 64
NCORES = 8
BPC = B // NCORES          # batches per core
NCH = N // 128             # 8 partition chunks of N
NJ = (T * F) // 128        # 12 chunks of the tf dim
NG = 3                     # t-groups
TT = T // NG               # 8 t's per group
NJG = NJ // NG             # 4 tf-chunks per t-group
XG = 3                     # x DMA groups
NJX = NJ // XG             # 4 tf-chunks per x DMA
bf16 = ml_dtypes.bfloat16
f8e4 = ml_dtypes.float8_e4m3

_CACHE = {}


def _build_nc():
    import concourse.bacc as bacc
    import concourse.bass as bass
    import concourse.tile as tile
    import concourse.mybir as mybir

    fp32 = mybir.dt.float32
    bf = mybir.dt.bfloat16
    f8 = mybir.dt.float8e4
    AF = mybir.ActivationFunctionType
    DR = mybir.MatmulPerfMode.DoubleRow

    nc = bacc.Bacc(
        "TRN2", target_bir_lowering=False, debug=False,
        enable_asserts=True, num_devices=NCORES,
    )

    # ---- DRAM I/O ----
    x_d = nc.dram_tensor("x_tf", [BPC, NJ, 128, N], bf, kind="ExternalInput")
    bs_d = nc.dram_tensor("bs_t", [NCH, 128, N], bf, kind="ExternalInput")
    vs8_d = nc.dram_tensor("vs8_t", [NCH, 128, N], f8, kind="ExternalInput")
    cheb_d = nc.dram_tensor("cheb_t", [NCH, 128, K, N], bf,
                            kind="ExternalInput")
    wcat_d = nc.dram_tensor("wcat", [128, NJ, 2 * T], bf, kind="ExternalInput")
    th2_d = nc.dram_tensor("th2", [128, 2 * K * FO], bf, kind="ExternalInput")
    thf_d = nc.dram_tensor("thf", [128, K * FO], bf, kind="ExternalInput")
    hrow_d = nc.dram_tensor("hrow", [NCH, 128, 1], fp32, kind="ExternalInput")
    id_d = nc.dram_tensor("id128", [128, 128], bf, kind="ExternalInput")
    # out[b, group, mchunk, p, o, tl]
    out_d = nc.dram_tensor("out", [BPC, NG, NCH, 128, FO, TT], bf,
                           kind="ExternalOutput")

    with tile.TileContext(nc) as tc:
        with (
            tc.tile_pool(name="const", bufs=1) as cpool,
            tc.tile_pool(name="work", bufs=2) as wpool,
            tc.tile_pool(name="big", bufs=1) as bpool,
            tc.tile_pool(name="ypool", bufs=2) as ypool,
            tc.tile_pool(name="chpool", bufs=3) as chpool,
            tc.tile_pool(name="spool", bufs=2) as spool,
            tc.tile_pool(name="psA", bufs=2, space="PSUM") as psA,
            tc.tile_pool(name="psB", bufs=2, space="PSUM") as psB,
        ):
            # ---- constants ----
            wcat_sb = cpool.tile([128, NJ, 2 * T], bf, tag="wcat")
            th2_sb = cpool.tile([128, 2 * K * FO], bf, tag="th2")
            vs8_sb = cpool.tile([128, NCH, N], f8, tag="vs8")
            bs_sb = cpool.tile([128, NCH, N], bf, tag="bs")
            hrow_sb = cpool.tile([128, NCH], fp32, tag="hrow")
            id_sb = cpool.tile([128, 128], bf, tag="id128")
            ones_sb = cpool.tile([128, 1], bf, tag="ones")
            one1_sb = cpool.tile([1, 1], fp32, tag="one1")
            warm_sb = cpool.tile([128, 512], bf, tag="warm")
            nc.gpsimd.memset(ones_sb[:], 1.0)
            nc.gpsimd.memset(one1_sb[:], 1.0)
            nc.gpsimd.memset(warm_sb[:], 0.25)

            # ---- PE warm-up: ~3.4us cold + a short warm tail so HAM
            # unthrottles right as x lands; more would gate the attention
            # matmuls behind the warm-up drain (results never consumed) ----
            for w in range(12):
                pw = psA.tile([128, 512], fp32, tag="big", name=f"warm{w}")
                nc.tensor.matmul(pw[:, :], warm_sb[:, 0:128], warm_sb[:, :],
                                 start=True, stop=True)

            def make_batch(b):
                st = {'ys': [], 'ch': [None] * NCH}

                def load_cheb(ic):
                    ch = chpool.tile([128, K, N], bf, tag="ch",
                                     name=f"ch{b}_{ic}")
                    st['ch'][ic] = ch
                    nc.sync.dma_start(ch[:], cheb_d[ic])

                def head_dma():
                    st['x'] = x_sb = bpool.tile([128, NJ, N], bf, tag="x",
                                                name=f"x{b}")
                    st['e'] = bpool.tile([128, NCH, N], bf, tag="e",
                                         name=f"e{b}")
                    st['p8'] = bpool.tile([128, NCH, N], f8, tag="p8",
                                          name=f"p{b}")
                    st['a'] = bpool.tile([128, K, NCH, N], bf, tag="a",
                                         name=f"a{b}")
                    st['rT'] = bpool.tile([128, NCH], fp32, tag="rT",
                                          name=f"rT{b}")
                    # DRAM-side APs rearranged so both sides' flattened dim
                    # orders match (DMA streams src sequence into dst sequence)
                    if b == 0:
                        nc.sync.dma_start(wcat_sb[:], wcat_d[:])
                    # first x group split in two so the attention matmuls
                    # start on the first half ~2us earlier
                    for h in range(2):
                        hj = NJX // 2
                        nc.sync.dma_start(
                            x_sb[:, h * hj:(h + 1) * hj, :],
                            x_d[b, h * hj:(h + 1) * hj].rearrange(
                                "j p n -> p j n"))
                    if b == 0:
                        nc.sync.dma_start(th2_sb[:], th2_d[:])
                    for g in range(1, XG):
                        nc.sync.dma_start(
                            x_sb[:, g * NJX:(g + 1) * NJX, :],
                            x_d[b, g * NJX:(g + 1) * NJX].rearrange(
                                "j p n -> p j n"))
                    if b == 0:
                        nc.sync.dma_start(bs_sb[:],
                                          bs_d[:].rearrange("c p n -> p c n"))
                        nc.sync.dma_start(vs8_sb[:],
                                          vs8_d[:].rearrange("c p n -> p c n"))
                        nc.sync.dma_start(
                            hrow_sb[:],
                            hrow_d[:].rearrange("c p one -> p (c one)"))
                        nc.sync.dma_start(id_sb[:], id_d[:])
                    for ic in range(3):
                        load_cheb(ic)

                def att_group(g):
                    # attention pre-reductions (one pass over x), emitted per
                    # x DMA group so Y-build units back-fill the DMA gaps
                    x_sb = st['x']
                    if g == 0:
                        st['att_c'] = wpool.tile([2 * T, N], bf,
                                                 tag="attc", bufs=1,
                                                 name=f"attc{b}")
                        st['att_r'] = wpool.tile([T, N], bf, tag="attr",
                                                 bufs=1, name=f"attr{b}")
                        st['pas'] = [psA.tile([2 * T, 512], fp32, tag="big",
                                              name=f"pa{b}_{s}")
                                     for s in range(2)]
                    pas = st['pas']
                    for j in range(NJX * g, NJX * (g + 1)):
                        for s in range(2):
                            nc.tensor.matmul(
                                pas[s][:, :],
                                wcat_sb[:, j, :],
                                x_sb[:, j, s * 512:(s + 1) * 512],
                                start=(j == 0), stop=(j == NJ - 1),
                            )

                def att_fin():
                    att_c, att_r, pas = st['att_c'], st['att_r'], st['pas']
                    # copies split across DVE/ACT (parallel), shift split per
                    # half so each product half starts as soon as its half
                    # of att_r lands
                    nc.vector.tensor_copy(att_c[:, 0:512], pas[0][:])
                    nc.scalar.copy(att_c[:, 512:1024], pas[1][:])
                    for s in range(2):
                        nc.sync.dma_start(
                            att_r[:, s * 512:(s + 1) * 512],
                            att_c[T:2 * T, s * 512:(s + 1) * 512])

                def product():
                    # bs accumulated into the product PSUM via an identity
                    # matmul: extra PE work lands exactly in the window the
                    # PE would otherwise starve, and the DVE stays free to
                    # drain Y copies; tanh reads PSUM directly
                    att_c, att_r, p8 = st['att_c'], st['att_r'], st['p8']
                    for cn in range(NCH):
                        for s in range(2):
                            pp = psA.tile([128, 512], fp32, tag="big",
                                          name=f"pp{b}")
                            nc.tensor.matmul(
                                pp[:, :],
                                att_c[0:T, cn * 128:(cn + 1) * 128],
                                att_r[:, s * 512:(s + 1) * 512],
                                start=True, stop=False,
                            )
                            nc.tensor.matmul(
                                pp[:, :],
                                id_sb[:],
                                bs_sb[:, cn, s * 512:(s + 1) * 512],
                                start=False, stop=True,
                            )
                            nc.scalar.activation(
                                p8[:, cn, s * 512:(s + 1) * 512], pp[:],
                                AF.Tanh, scale=0.5)

                def y_pair(y_sb, j, tl0, cnp, eng):
                    # two Y units (adjacent cn) into one 2-bank PSUM tile,
                    # drained by a single copy: halves the per-element copy
                    # overhead that paces the whole head phase
                    x_sb = st['x']
                    cn0 = 2 * cnp
                    W = 2 * K * FO
                    py2 = psB.tile([128, 2, 512], fp32, tag="py2", bufs=2,
                                   name=f"py{b}")
                    for u in range(2):
                        csl = slice((cn0 + u) * 128, (cn0 + u + 1) * 128)
                        nc.tensor.matmul(
                            py2[:, u, 0:W], x_sb[:, j, csl], th2_sb[:],
                            start=True, stop=True,
                        )
                    src = py2[:, :, 0:W].rearrange(
                        "p u (k w) -> p u k w", k=K)
                    dst = y_sb[:, cn0:cn0 + 2, :, tl0:tl0 + 2, :].rearrange(
                        "p c k t o -> p c k (t o)")
                    if eng == 'd':
                        nc.vector.tensor_copy(dst, src)
                    else:
                        nc.scalar.copy(dst, src)

                def build_y(g, pat):
                    # pat: cycled per-pair copy-engine pattern, 'd'=DVE 'a'=ACT
                    y_sb = ypool.tile([128, NCH, K, TT, FO], bf, tag="y",
                                      name=f"y{b}_{g}")
                    st['ys'].append(y_sb)
                    i = 0
                    for j in range(NJG * g, NJG * (g + 1)):
                        for cnp in range(NCH // 2):
                            y_pair(y_sb, j, 2 * (j - NJG * g), cnp,
                                   pat[i % len(pat)])
                            i += 1

                def sphase():
                    e_sb, p8, a_sb = st['e'], st['p8'], st['a']
                    for ic in range(NCH):
                        for s in range(2):
                            ps = psA.tile([128, 512], fp32, tag="big",
                                          name=f"ps{b}")
                            for j2 in range(NCH // 2):
                                nc.tensor.matmul(
                                    ps[:, :],
                                    vs8_sb[:, 2 * j2:2 * j2 + 2,
                                           ic * 128:(ic + 1) * 128],
                                    p8[:, 2 * j2:2 * j2 + 2,
                                       s * 512:(s + 1) * 512],
                                    start=(j2 == 0), stop=(j2 == NCH // 2 - 1),
                                    perf_mode=DR,
                                )
                            nc.scalar.activation(
                                e_sb[:, ic, s * 512:(s + 1) * 512], ps[:],
                                AF.Exp, scale=0.5,
                                bias=hrow_sb[:, ic:ic + 1],
                            )
                        ch = st['ch'][ic]
                        for k in range(K):
                            nc.vector.tensor_mul(a_sb[:, k, ic, :],
                                                 ch[:, k, :],
                                                 e_sb[:, ic, :])
                        if ic + 3 < NCH:
                            load_cheb(ic + 3)

                def rt():
                    e_sb, rT_sb = st['e'], st['rT']
                    pcs = [psA.tile([1, 512], fp32, tag="big",
                                    name=f"pc{b}_{s}") for s in range(2)]
                    for ic in range(NCH):
                        for s in range(2):
                            nc.tensor.matmul(
                                pcs[s][:, :],
                                ones_sb[:],
                                e_sb[:, ic, s * 512:(s + 1) * 512],
                                start=(ic == 0), stop=(ic == NCH - 1),
                            )
                    csum_sb = wpool.tile([1, N], fp32, tag="csum_s", bufs=1,
                                         name=f"cs{b}")
                    for s in range(2):
                        nc.scalar.copy(csum_sb[:, s * 512:(s + 1) * 512],
                                       pcs[s][:])
                    prt = psA.tile([128, NCH], fp32, tag="big",
                                   name=f"prt{b}")
                    for c in range(NCH):
                        nc.tensor.matmul(
                            prt[:, c:c + 1],
                            csum_sb[:, c * 128:(c + 1) * 128],
                            one1_sb[:],
                            start=True, stop=True,
                        )
                    nc.vector.reciprocal(rT_sb[:], prt[:])

                def conv(g, tail=False):
                    a_sb, rT_sb, y_sb = st['a'], st['rT'], st['ys'][g]
                    for mh in range(2):
                        stage = spool.tile([128, NCH // 2, FO, TT], bf,
                                           tag="stage", name=f"st{b}")
                        fine = tail and mh == 1
                        for mi in range(NCH // 2):
                            mc = mh * (NCH // 2) + mi
                            po = psB.tile([128, TT, FO], fp32, tag="po",
                                          bufs=2, name=f"po{b}")
                            nmm = 0
                            for k in range(K):
                                for cn in range(NCH):
                                    nc.tensor.matmul(
                                        po[:, :, :],
                                        a_sb[:, k, cn,
                                             mc * 128:(mc + 1) * 128],
                                        y_sb[:, cn, k, :, :],
                                        start=(nmm == 0),
                                        stop=(nmm == K * NCH - 1),
                                    )
                                    nmm += 1
                            nc.scalar.activation(
                                stage[:, mi],
                                po[:, :, :].rearrange("p t o -> p o t"),
                                AF.Relu,
                                scale=rT_sb[:, mc:mc + 1],
                            )
                            if fine:
                                # kernel tail: ship each mc as soon as its
                                # relu lands instead of waiting for the
                                # whole half-group
                                nc.sync.dma_start(out_d[b, g, mc],
                                                  stage[:, mi])
                        if fine:
                            continue
                        # store on the sync queue (idle mid-kernel); issuing
                        # from ACT would stall the relu copies that recycle
                        # conv's PSUM slots
                        nc.sync.dma_start(
                            out_d[b, g, mh * (NCH // 2):(mh + 1) * (NCH // 2)]
                            .rearrange("m p o t -> p m o t"),
                            stage[:])

                st['head_dma'] = head_dma
                st['att_group'] = att_group
                st['att_fin'] = att_fin
                st['product'] = product
                st['build_y'] = build_y
                st['sphase'] = sphase
                st['rt'] = rt
                st['conv'] = conv
                return st

            # ---- emission order (= scheduler priority); the priority heap
            # back-fills any stall with ready lower-priority work ----
            # y slots (2) and the shared 6-slot PSUM ring are recycled in
            # emission order, so each build_y is emitted after the conv
            # that releases its slot; the priority heap back-fills stalls
            # with whatever is ready.
            b0 = make_batch(0)
            b0['head_dma']()
            b0['att_group'](0)
            b0['build_y'](0, 'da')
            b0['att_group'](1)
            b0['att_group'](2)
            b0['att_fin']()
            b0['product']()
            b0['build_y'](1, 'dda')   # PE filler under the add/tanh chain
            b0['sphase']()
            b0['rt']()
            b1 = make_batch(1)
            b1['head_dma']()
            b0['conv'](0)
            b0['build_y'](2, 'd')
            for g in range(XG):
                b1['att_group'](g)
            b1['att_fin']()
            b0['conv'](1)
            b1['product']()
            b1['build_y'](0, 'd')
            b0['conv'](2)
            b1['sphase']()
            b1['build_y'](1, 'da')    # split drain: DVE is full of a-muls
            b1['rt']()
            b1['conv'](0)
            b1['build_y'](2, 'd')
            b1['conv'](1)
            b1['conv'](2, tail=True)

    nc.compile()
    return nc


def _host_prep(x, W1, W2, W3, bs, Vs, cheb, Theta):
    x = np.asarray(x, np.float32)
    W1 = np.asarray(W1, np.float32)
    W2 = np.asarray(W2, np.float32)
    W3 = np.asarray(W3, np.float32)
    bs = np.asarray(bs, np.float32)
    Vs = np.asarray(Vs, np.float32)
    cheb = np.asarray(cheb, np.float32)
    Theta = np.asarray(Theta, np.float32)

    x_tf = np.ascontiguousarray(x.transpose(0, 3, 2, 1)).reshape(B, NJ, 128, N)
    x_tf = x_tf.astype(bf16)
    bs_t = bs[0].reshape(NCH, 128, N).astype(bf16)
    vs8_t = np.ascontiguousarray(Vs.T).reshape(NCH, 128, N).astype(f8e4)
    cheb_t = np.ascontiguousarray(
        cheb.reshape(K, NCH, 128, N).transpose(1, 2, 0, 3)).astype(bf16)
    t_idx = np.arange(T * F) // F
    f_idx = np.arange(T * F) % F
    wl_flat = W1[t_idx][:, None] * W2[f_idx, :]
    wr_flat = np.zeros((T * F, T), np.float32)
    wr_flat[np.arange(T * F), t_idx] = W3[f_idx]
    wcat = np.concatenate([wl_flat, wr_flat], axis=1)
    # partition-major so the single DMA moves 2.3KB contiguous per partition
    wcat = np.ascontiguousarray(
        wcat.reshape(NJ, 128, 2 * T).transpose(1, 0, 2)).astype(bf16)
    # columns ordered (k, t-parity, o) so each Y pair drains with one
    # strided copy
    th2 = np.zeros((128, 2 * K * FO), np.float32)
    for par in range(2):
        for k in range(K):
            c0 = k * 2 * FO + par * FO
            th2[par * F:(par + 1) * F, c0:c0 + FO] = Theta[k]
    th2 = th2.astype(bf16)
    # f-contraction variant: both partition halves hold Theta, columns (k, o)
    thf = np.zeros((128, K * FO), np.float32)
    for par in range(2):
        for k in range(K):
            thf[par * F:(par + 1) * F, k * FO:(k + 1) * FO] = Theta[k]
    thf = thf.astype(bf16)
    hrow = (0.5 * Vs.sum(axis=1)).astype(np.float32).reshape(NCH, 128, 1)
    id128 = np.eye(128, dtype=np.float32).astype(bf16)
    return x_tf, bs_t, vs8_t, cheb_t, wcat, th2, thf, hrow, id128


def kernel(x, W1, W2, W3, bs, Vs, cheb, Theta, _return_results=False,
           _trace=False):
    from concourse.bass_utils import run_bass_kernel_spmd

    x_tf, bs_t, vs8_t, cheb_t, wcat, th2, thf, hrow, id128 = _host_prep(
        x, W1, W2, W3, bs, Vs, cheb, Theta)

    if "nc" not in _CACHE:
        _CACHE["nc"] = _build_nc()
    nc = _CACHE["nc"]

    shared = dict(bs_t=bs_t, vs8_t=vs8_t, cheb_t=cheb_t, wcat=wcat,
                  th2=th2, thf=thf, hrow=hrow, id128=id128)
    in_maps = []
    for c in range(NCORES):
        m = dict(shared)
        m["x_tf"] = np.ascontiguousarray(x_tf[c * BPC:(c + 1) * BPC])
        in_maps.append(m)

    _CACHE["in_maps"] = in_maps
    kw = {"trace": True} if _trace else {}
    res = run_bass_kernel_spmd(nc, in_maps, list(range(NCORES)), **kw)
    outs = []
    for c in range(NCORES):
        o = res.results[c]["out"]  # (BPC, NG, NCH, 128, FO, TT)
        o = np.asarray(o, np.float32)
        o = o.transpose(0, 2, 3, 4, 1, 5).reshape(BPC, N, FO, T)
        outs.append(o)
    full = np.concatenate(outs, axis=0).astype(np.float32)
    if _return_results:
        return full, res
    return full
